# revision 6
# baseline (speedup 1.0000x reference)
"""Cross-attention kernel for Trainium2, SPMD over 8 NeuronCores.

Problem: B=2, LQ=1024, LK=10000, E=256, H=8 heads of D=32.
  q = queries @ Wq + bq ; k = bev @ Wk + bk ; v = bev @ Wv + bv
  out = softmax(q k^T) v  @ Wo + bo

Sharding: core c -> (batch b = c // 4, head-pair hp = c % 4).  Each core
computes attention for its 2 heads of its batch plus the partial output
projection through its 64 rows of Wo.  Host sums the 4 partials per batch
and adds bo (the gather/unshard step).  No collectives.

Numerics: big matmuls run as float32r (full-rate fp32, ~2^-11 rounding on
inputs, fp32 accumulate).  Softmax skips the max-subtraction (energies are
~N(0,32); exp stays finite in fp32) and the denominator is produced by an
extra all-ones column appended to v, so it falls out of the same PE matmul
that computes attn @ v.  bv is mathematically equivalent to a constant
added after normalization (softmax weights sum to 1), so v is projected
without bias and bv is added at the end.
"""
import sys

sys.path.insert(0, "/opt/trn_rl_repo")

import numpy as np

B, LQ, LK, E, H = 2, 1024, 10000, 256, 8
D = 32            # head dim
HPC = 2           # heads per core
DC = D * HPC      # 64 projected dims per core
LKP = 10240       # LK padded to a multiple of 128
NKT = LKP // 128  # 80 k-tiles
NCH = LKP // 512  # 20 dma/transpose chunks
GRP = 3           # (kt, head) units per exp instruction ([128, 1536] staging)

_CACHE = {}


def _build():
    import concourse.bacc as bacc
    import concourse.tile as tile
    from concourse import mybir

    FP32 = mybir.dt.float32
    FP32R = mybir.dt.float32r
    AF = mybir.ActivationFunctionType

    nc = bacc.Bacc("TRN2", target_bir_lowering=False)

    XQ = nc.dram_tensor("xq", [LQ, E], FP32, kind="ExternalInput")
    XK = nc.dram_tensor("xk", [LKP, E], FP32, kind="ExternalInput")
    WQ = nc.dram_tensor("wq", [E, DC], FP32, kind="ExternalInput")
    WK = nc.dram_tensor("wk", [E, DC], FP32, kind="ExternalInput")
    WV = nc.dram_tensor("wv", [E, DC], FP32, kind="ExternalInput")
    WO = nc.dram_tensor("wo", [DC, E], FP32, kind="ExternalInput")
    BQ = nc.dram_tensor("bq", [DC], FP32, kind="ExternalInput")
    BK = nc.dram_tensor("bk", [DC], FP32, kind="ExternalInput")
    BV = nc.dram_tensor("bv", [DC], FP32, kind="ExternalInput")
    IDT = nc.dram_tensor("ident", [128, 128], FP32, kind="ExternalInput")
    # partial output, transposed: rows = embed dim, cols = query position
    OUT = nc.dram_tensor("out_t", [E, LQ], FP32, kind="ExternalOutput")

    with tile.TileContext(nc) as tc:
        with (
            tc.tile_pool(name="singles", bufs=1) as sg,
            tc.tile_pool(name="aio", bufs=3) as aio,
            tc.tile_pool(name="wk", bufs=3) as wkp,
            tc.tile_pool(name="ps", bufs=2, space="PSUM") as ps,
            tc.tile_pool(name="av", bufs=2, space="PSUM") as avp,
        ):
            # ---- constants ----
            ident = sg.tile([128, 128], FP32, tag="ident")
            nc.sync.dma_start(out=ident, in_=IDT[:, :])

            ones = sg.tile([128, 160], FP32, tag="ones")
            nc.vector.memset(ones, 1.0)

            def load_round(dram_ap, shape, tag):
                f = aio.tile(shape, FP32, tag="wstage", name="wstage")
                nc.sync.dma_start(out=f, in_=dram_ap)
                r = sg.tile(shape, FP32R, tag=tag, name=tag)
                nc.vector.tensor_copy(r, f)
                return r

            wq_r = load_round(WQ[:, :].rearrange("(c p) m -> p c m", p=128),
                              [128, 2, DC], "wq")
            wk_r = load_round(WK[:, :].rearrange("(c p) m -> p c m", p=128),
                              [128, 2, DC], "wk")
            wv_r = load_round(WV[:, :].rearrange("(c p) m -> p c m", p=128),
                              [128, 2, DC], "wv")

            # Wo rows: head h's 32 rows at partitions 0-31, column block h
            wo_f = aio.tile([32, 2, E], FP32, tag="wstage2")
            nc.sync.dma_start(
                out=wo_f, in_=WO[:, :].rearrange("(a p) m -> p a m", p=32))
            wo_r = sg.tile([32, 2, E], FP32R, tag="wo")
            nc.vector.tensor_copy(wo_r, wo_f)

            bq_sb = sg.tile([64, 1], FP32, tag="bq")
            nc.sync.dma_start(out=bq_sb, in_=BQ[:].rearrange("(p o) -> p o", o=1))
            bk_sb = sg.tile([64, 1], FP32, tag="bk")
            nc.sync.dma_start(out=bk_sb, in_=BK[:].rearrange("(p o) -> p o", o=1))
            # bv: column h holds head h's 32 bias values on partitions 0-31
            bv_sb = sg.tile([32, 2], FP32, tag="bv")
            nc.sync.dma_start(out=bv_sb,
                              in_=BV[:].rearrange("(a p) -> p a", p=32))

            # ---- stage A-q: transpose queries, project q^T ----
            xqT = [sg.tile([128, LQ], FP32R, tag=f"xqT{e}", name=f"xqT{e}") for e in range(2)]
            for half in range(2):          # groups of 4 q-tiles
                pts = [ps.tile([128, 512], FP32, tag="stg", name=f"ptsq{half}{i}") for i in range(2)]
                for j in range(4):
                    qt = half * 4 + j
                    xq_t = aio.tile([128, E], FP32, tag="xq")
                    nc.sync.dma_start(out=xq_t, in_=XQ[qt * 128:(qt + 1) * 128, :])
                    for e in range(2):
                        nc.tensor.transpose(
                            pts[e][:, j * 128:(j + 1) * 128],
                            xq_t[:, e * 128:(e + 1) * 128], ident)
                for e in range(2):
                    nc.vector.tensor_copy(
                        xqT[e][:, half * 512:(half + 1) * 512], pts[e])

            qT = sg.tile([64, LQ], FP32R, tag="qT")
            for qc in range(2):
                qp = ps.tile([64, 512], FP32, tag="stg")
                for e in range(2):
                    nc.tensor.matmul(qp, wq_r[:, e, :],
                                     xqT[e][:, qc * 512:(qc + 1) * 512],
                                     start=(e == 0), stop=(e == 1))
                nc.vector.tensor_scalar_add(
                    qT[:, qc * 512:(qc + 1) * 512], qp, bq_sb[:, 0:1])

            # ---- stage A-k: transpose bev chunks, project k^T and v ----
            kT = sg.tile([64, LKP], FP32R, tag="kT")
            v_aug = sg.tile([128, NKT * 66], FP32R, tag="vaug")
            # ones columns of v_aug (the softmax-denominator trick)
            nc.vector.tensor_copy(
                v_aug[:, :].rearrange("p (k o) -> p k o", o=33)[:, :, 32:33],
                ones[:, :].rearrange("p (k o) -> p k o", o=1))

            for c in range(NCH):
                xk_t = aio.tile([128, 4, E], FP32, tag="xk")
                nc.sync.dma_start(
                    out=xk_t,
                    in_=XK[c * 512:(c + 1) * 512, :].rearrange(
                        "(t p) e -> p t e", p=128))
                xkT = []
                for e in range(2):
                    pt = ps.tile([128, 512], FP32, tag="stg")
                    for t in range(4):
                        nc.tensor.transpose(
                            pt[:, t * 128:(t + 1) * 128],
                            xk_t[:, t, e * 128:(e + 1) * 128], ident)
                    xe = wkp.tile([128, 512], FP32R, tag=f"xkT{e}")
                    nc.vector.tensor_copy(xe, pt)
                    xkT.append(xe)

                kp = ps.tile([64, 512], FP32, tag="stg")
                for e in range(2):
                    nc.tensor.matmul(kp, wk_r[:, e, :], xkT[e],
                                     start=(e == 0), stop=(e == 1))
                nc.vector.tensor_scalar_add(
                    kT[:, c * 512:(c + 1) * 512], kp, bk_sb[:, 0:1])

                vp = ps.tile([64, 512], FP32, tag="stg")
                for e in range(2):
                    nc.tensor.matmul(vp, wv_r[:, e, :], xkT[e],
                                     start=(e == 0), stop=(e == 1))
                vt = wkp.tile([64, 512], FP32, tag="vt")
                nc.vector.tensor_copy(vt, vp)

                vps = ps.tile([128, 256], FP32, tag="stg")
                for t in range(4):
                    nc.tensor.transpose(vps[:, t * 64:(t + 1) * 64],
                                        vt[:, t * 128:(t + 1) * 128],
                                        ident[0:64, 0:64])
                nc.vector.tensor_copy(
                    v_aug[:, c * 264:(c + 1) * 264].rearrange(
                        "p (t a b) -> p t a b", a=2, b=33)[:, :, :, 0:32],
                    vps[:, :].rearrange("p (t a b) -> p t a b", a=2, b=32))

            # ---- stage B + C per query half ----
            # attnT: head h's normalized attn output (32 dims) on partitions
            # 0-31, columns h*LQ + q.  recip row 32 holds 1/denominator.
            attnT = sg.tile([32, 2 * LQ], FP32R, tag="attnT")
            recip = sg.tile([33, 2 * LQ], FP32, tag="recip")
            out_sb = [sg.tile([128, LQ], FP32, tag=f"out{e}", name=f"out{e}") for e in range(2)]

            units = [(kt, h) for kt in range(NKT) for h in range(HPC)]
            for qc in range(2):
                qs = slice(qc * 512, (qc + 1) * 512)
                av = [avp.tile([33, 512], FP32, tag="av", name=f"av{qc}{h}")
                      for h in range(HPC)]
                for g0 in range(0, len(units), GRP):
                    grp = units[g0:g0 + GRP]
                    stg = ps.tile([128, 512 * len(grp)], FP32, tag="stg")
                    for i, (kt, h) in enumerate(grp):
                        nc.tensor.matmul(
                            stg[:, i * 512:(i + 1) * 512],
                            kT[32 * h:32 * h + 32, kt * 128:(kt + 1) * 128],
                            qT[32 * h:32 * h + 32, qs],
                            start=True, stop=True)
                    sT = wkp.tile([128, 512 * len(grp)], FP32R, tag="sT")
                    nc.scalar.activation(sT, stg, AF.Exp)
                    for i, (kt, h) in enumerate(grp):
                        nc.tensor.matmul(
                            av[h][0:33, :],
                            v_aug[:, kt * 66 + 33 * h:kt * 66 + 33 * h + 33],
                            sT[:, i * 512:(i + 1) * 512],
                            start=(kt == 0), stop=(kt == NKT - 1),
                            skip_group_check=True)

                # normalize + bias + output projection
                rb = ps.tile([32, 1024], FP32, tag="stg")
                for h in range(HPC):
                    hqs = slice(h * LQ + qc * 512, h * LQ + (qc + 1) * 512)
                    nc.vector.reciprocal(recip[32:33, hqs], av[h][32:33, :])
                    nc.tensor.matmul(rb[0:32, h * 512:(h + 1) * 512],
                                     ones[32:33, 0:32], recip[32:33, hqs],
                                     start=True, stop=True,
                                     tile_position=(32, 0))
                rbs = wkp.tile([32, 1024], FP32, tag="rbs")
                nc.vector.tensor_copy(rbs, rb)
                tmp = wkp.tile([32, 1024], FP32, tag="ctmp")
                for h in range(HPC):
                    hqs = slice(h * LQ + qc * 512, h * LQ + (qc + 1) * 512)
                    nc.vector.tensor_mul(tmp[:, h * 512:(h + 1) * 512],
                                         av[h][0:32, :],
                                         rbs[:, h * 512:(h + 1) * 512])
                    nc.vector.tensor_scalar_add(attnT[0:32, hqs],
                                                tmp[:, h * 512:(h + 1) * 512],
                                                bv_sb[:, h:h + 1])

                for ec in range(2):
                    po = ps.tile([128, 512], FP32, tag="stg")
                    for h in range(HPC):
                        hqs = slice(h * LQ + qc * 512, h * LQ + (qc + 1) * 512)
                        nc.tensor.matmul(
                            po, wo_r[:, h, ec * 128:(ec + 1) * 128],
                            attnT[0:32, hqs], start=(h == 0), stop=(h == 1))
                    nc.vector.tensor_copy(out_sb[ec][:, qs], po)

            for ec in range(2):
                nc.sync.dma_start(out=OUT[ec * 128:(ec + 1) * 128, :],
                                  in_=out_sb[ec])

    nc.compile()
    return nc


def _get_nc():
    if "nc" not in _CACHE:
        _CACHE["nc"] = _build()
    return _CACHE["nc"]


def kernel(bev_emb, queries, Wq, bq, Wk, bk, Wv, bv, Wo, bo):
    from concourse.bass_utils import run_bass_kernel_spmd

    bev_emb = np.asarray(bev_emb, dtype=np.float32)
    queries = np.asarray(queries, dtype=np.float32)
    Wq = np.asarray(Wq, dtype=np.float32)
    bq = np.asarray(bq, dtype=np.float32)
    Wk = np.asarray(Wk, dtype=np.float32)
    bk = np.asarray(bk, dtype=np.float32)
    Wv = np.asarray(Wv, dtype=np.float32)
    bv = np.asarray(bv, dtype=np.float32)
    Wo = np.asarray(Wo, dtype=np.float32)
    bo = np.asarray(bo, dtype=np.float32)

    xk_pad = np.zeros((B, LKP, E), dtype=np.float32)
    xk_pad[:, :LK, :] = bev_emb
    ident = np.eye(128, dtype=np.float32)

    in_maps = []
    for c in range(8):
        b, hp = c // 4, c % 4
        hs = slice(hp * DC, (hp + 1) * DC)
        in_maps.append({
            "xq": np.ascontiguousarray(queries[b]),
            "xk": np.ascontiguousarray(xk_pad[b]),
            "wq": np.ascontiguousarray(Wq[:, hs]),
            "wk": np.ascontiguousarray(Wk[:, hs]),
            "wv": np.ascontiguousarray(Wv[:, hs]),
            "wo": np.ascontiguousarray(Wo[hs, :]),
            "bq": np.ascontiguousarray(bq[hs]),
            "bk": np.ascontiguousarray(bk[hs]),
            "bv": np.ascontiguousarray(bv[hs]),
            "ident": ident,
        })

    nc = _get_nc()
    _CACHE["last_in_maps"] = in_maps
    res = run_bass_kernel_spmd(nc, in_maps, list(range(8)))
    _CACHE["last_result"] = res

    out = np.zeros((B, LQ, E), dtype=np.float32)
    for c in range(8):
        out[c // 4] += res.results[c]["out_t"].T
    out += bo
    return out


# revision 7
# speedup vs baseline: 1.0076x; 1.0076x over previous
"""Cross-attention kernel for Trainium2, SPMD over 8 NeuronCores.

Problem: B=2, LQ=1024, LK=10000, E=256, H=8 heads of D=32.
  q = queries @ Wq + bq ; k = bev @ Wk + bk ; v = bev @ Wv + bv
  out = softmax(q k^T) v  @ Wo + bo

Sharding: core c -> (batch b = c // 4, head-pair hp = c % 4).  Each core
computes attention for its 2 heads of its batch plus the partial output
projection through its 64 rows of Wo.  Host sums the 4 partials per batch
and adds bo (the gather/unshard step).  No collectives.

Numerics: big matmuls run as float32r (full-rate fp32, ~2^-11 rounding on
inputs, fp32 accumulate).  Softmax skips the max-subtraction (energies are
~N(0,32); exp stays finite in fp32) and the denominator is produced by an
extra all-ones column appended to v, so it falls out of the same PE matmul
that computes attn @ v.  bv is mathematically equivalent to a constant
added after normalization (softmax weights sum to 1), so v is projected
without bias and bv is added at the end.
"""
import sys

sys.path.insert(0, "/opt/trn_rl_repo")

import numpy as np

B, LQ, LK, E, H = 2, 1024, 10000, 256, 8
D = 32            # head dim
HPC = 2           # heads per core
DC = D * HPC      # 64 projected dims per core
LKP = 10240       # LK padded to a multiple of 128
NKT = LKP // 128  # 80 k-tiles
NCH = LKP // 512  # 20 dma/transpose chunks
GRP = 3           # (kt, head) units per exp instruction ([128, 1536] staging)

_CACHE = {}


def _build():
    import concourse.bacc as bacc
    import concourse.tile as tile
    from concourse import mybir

    FP32 = mybir.dt.float32
    FP32R = mybir.dt.float32r
    BF16 = mybir.dt.bfloat16
    AF = mybir.ActivationFunctionType

    nc = bacc.Bacc("TRN2", target_bir_lowering=False)

    XQ = nc.dram_tensor("xq", [LQ, E], FP32, kind="ExternalInput")
    XK = nc.dram_tensor("xk", [LKP, E], FP32, kind="ExternalInput")
    WQ = nc.dram_tensor("wq", [E, DC], FP32, kind="ExternalInput")
    WK = nc.dram_tensor("wk", [E, DC], FP32, kind="ExternalInput")
    WV = nc.dram_tensor("wv", [E, DC], FP32, kind="ExternalInput")
    WO = nc.dram_tensor("wo", [DC, E], FP32, kind="ExternalInput")
    BQ = nc.dram_tensor("bq", [DC], FP32, kind="ExternalInput")
    BK = nc.dram_tensor("bk", [DC], FP32, kind="ExternalInput")
    BV = nc.dram_tensor("bv", [DC], FP32, kind="ExternalInput")
    IDT = nc.dram_tensor("ident", [128, 128], FP32, kind="ExternalInput")
    # partial output, transposed: rows = embed dim, cols = query position
    OUT = nc.dram_tensor("out_t", [E, LQ], FP32, kind="ExternalOutput")

    with tile.TileContext(nc) as tc:
        with (
            tc.tile_pool(name="singles", bufs=1) as sg,
            tc.tile_pool(name="aio", bufs=3) as aio,
            tc.tile_pool(name="wk", bufs=3) as wkp,
            tc.tile_pool(name="ps", bufs=2, space="PSUM") as ps,
            tc.tile_pool(name="av", bufs=2, space="PSUM") as avp,
        ):
            # ---- constants ----
            ident = sg.tile([128, 128], FP32, tag="ident")
            nc.sync.dma_start(out=ident, in_=IDT[:, :])

            ones = sg.tile([128, 160], FP32, tag="ones")
            nc.vector.memset(ones, 1.0)

            def load_round(dram_ap, shape, tag):
                f = aio.tile(shape, FP32, tag="wstage", name="wstage")
                nc.sync.dma_start(out=f, in_=dram_ap)
                r = sg.tile(shape, FP32R, tag=tag, name=tag)
                nc.vector.tensor_copy(r, f)
                return r

            wq_r = load_round(WQ[:, :].rearrange("(c p) m -> p c m", p=128),
                              [128, 2, DC], "wq")
            wk_r = load_round(WK[:, :].rearrange("(c p) m -> p c m", p=128),
                              [128, 2, DC], "wk")
            wv_r = load_round(WV[:, :].rearrange("(c p) m -> p c m", p=128),
                              [128, 2, DC], "wv")

            # Wo rows: head h's 32 rows at partitions 0-31, column block h
            wo_f = aio.tile([32, 2, E], FP32, tag="wstage2")
            nc.sync.dma_start(
                out=wo_f, in_=WO[:, :].rearrange("(a p) m -> p a m", p=32))
            wo_r = sg.tile([32, 2, E], FP32R, tag="wo")
            nc.vector.tensor_copy(wo_r, wo_f)

            bq_sb = sg.tile([64, 1], FP32, tag="bq")
            nc.sync.dma_start(out=bq_sb, in_=BQ[:].rearrange("(p o) -> p o", o=1))
            bk_sb = sg.tile([64, 1], FP32, tag="bk")
            nc.sync.dma_start(out=bk_sb, in_=BK[:].rearrange("(p o) -> p o", o=1))
            # bv: column h holds head h's 32 bias values on partitions 0-31
            bv_sb = sg.tile([32, 2], FP32, tag="bv")
            nc.sync.dma_start(out=bv_sb,
                              in_=BV[:].rearrange("(a p) -> p a", p=32))

            # ---- stage A-q: transpose queries, project q^T ----
            xqT = [sg.tile([128, LQ], FP32R, tag=f"xqT{e}", name=f"xqT{e}") for e in range(2)]
            for half in range(2):          # groups of 4 q-tiles
                pts = [ps.tile([128, 512], FP32, tag="stg", name=f"ptsq{half}{i}") for i in range(2)]
                for j in range(4):
                    qt = half * 4 + j
                    xq_t = aio.tile([128, E], FP32, tag="xq")
                    nc.sync.dma_start(out=xq_t, in_=XQ[qt * 128:(qt + 1) * 128, :])
                    for e in range(2):
                        nc.tensor.transpose(
                            pts[e][:, j * 128:(j + 1) * 128],
                            xq_t[:, e * 128:(e + 1) * 128], ident)
                for e in range(2):
                    nc.vector.tensor_copy(
                        xqT[e][:, half * 512:(half + 1) * 512], pts[e])

            qT = sg.tile([64, LQ], FP32R, tag="qT")
            for qc in range(2):
                qp = ps.tile([64, 512], FP32, tag="stg")
                for e in range(2):
                    nc.tensor.matmul(qp, wq_r[:, e, :],
                                     xqT[e][:, qc * 512:(qc + 1) * 512],
                                     start=(e == 0), stop=(e == 1))
                nc.vector.tensor_scalar_add(
                    qT[:, qc * 512:(qc + 1) * 512], qp, bq_sb[:, 0:1])

            # ---- stage A-k: transpose bev chunks, project k^T and v ----
            kT = sg.tile([64, LKP], FP32R, tag="kT")
            v_aug = sg.tile([128, NKT * 66], BF16, tag="vaug")
            # ones columns of v_aug (the softmax-denominator trick)
            nc.vector.tensor_copy(
                v_aug[:, :].rearrange("p (k o) -> p k o", o=33)[:, :, 32:33],
                ones[:, :].rearrange("p (k o) -> p k o", o=1))

            for c in range(NCH):
                xk_t = aio.tile([128, 4, E], FP32, tag="xk")
                nc.sync.dma_start(
                    out=xk_t,
                    in_=XK[c * 512:(c + 1) * 512, :].rearrange(
                        "(t p) e -> p t e", p=128))
                xkT = []
                for e in range(2):
                    pt = ps.tile([128, 512], FP32, tag="stg")
                    for t in range(4):
                        nc.tensor.transpose(
                            pt[:, t * 128:(t + 1) * 128],
                            xk_t[:, t, e * 128:(e + 1) * 128], ident)
                    xe = wkp.tile([128, 512], FP32R, tag=f"xkT{e}")
                    nc.vector.tensor_copy(xe, pt)
                    xkT.append(xe)

                kp = ps.tile([64, 512], FP32, tag="stg")
                for e in range(2):
                    nc.tensor.matmul(kp, wk_r[:, e, :], xkT[e],
                                     start=(e == 0), stop=(e == 1))
                nc.vector.tensor_scalar_add(
                    kT[:, c * 512:(c + 1) * 512], kp, bk_sb[:, 0:1])

                vp = ps.tile([64, 512], FP32, tag="stg")
                for e in range(2):
                    nc.tensor.matmul(vp, wv_r[:, e, :], xkT[e],
                                     start=(e == 0), stop=(e == 1))
                vt = wkp.tile([64, 512], FP32, tag="vt")
                nc.vector.tensor_copy(vt, vp)

                vps = ps.tile([128, 256], FP32, tag="stg")
                for t in range(4):
                    nc.tensor.transpose(vps[:, t * 64:(t + 1) * 64],
                                        vt[:, t * 128:(t + 1) * 128],
                                        ident[0:64, 0:64])
                nc.vector.tensor_copy(
                    v_aug[:, c * 264:(c + 1) * 264].rearrange(
                        "p (t a b) -> p t a b", a=2, b=33)[:, :, :, 0:32],
                    vps[:, :].rearrange("p (t a b) -> p t a b", a=2, b=32))

            # ---- stage B + C per query half ----
            # attnT: head h's normalized attn output (32 dims) on partitions
            # 0-31, columns h*LQ + q.  recip row 32 holds 1/denominator.
            attnT = sg.tile([32, 2 * LQ], FP32R, tag="attnT")
            recip = sg.tile([33, 2 * LQ], FP32, tag="recip")
            out_sb = [sg.tile([128, LQ], FP32, tag=f"out{e}", name=f"out{e}") for e in range(2)]

            units = [(kt, h) for kt in range(NKT) for h in range(HPC)]
            for qc in range(2):
                qs = slice(qc * 512, (qc + 1) * 512)
                av = [avp.tile([33, 512], FP32, tag="av", name=f"av{qc}{h}")
                      for h in range(HPC)]
                for g0 in range(0, len(units), GRP):
                    grp = units[g0:g0 + GRP]
                    stg = ps.tile([128, 512 * len(grp)], FP32, tag="stg")
                    for i, (kt, h) in enumerate(grp):
                        nc.tensor.matmul(
                            stg[:, i * 512:(i + 1) * 512],
                            kT[32 * h:32 * h + 32, kt * 128:(kt + 1) * 128],
                            qT[32 * h:32 * h + 32, qs],
                            start=True, stop=True)
                    sT = wkp.tile([128, 512 * len(grp)], BF16, tag="sT")
                    nc.scalar.activation(sT, stg, AF.Exp)
                    for i, (kt, h) in enumerate(grp):
                        nc.tensor.matmul(
                            av[h][0:33, :],
                            v_aug[:, kt * 66 + 33 * h:kt * 66 + 33 * h + 33],
                            sT[:, i * 512:(i + 1) * 512],
                            start=(kt == 0), stop=(kt == NKT - 1),
                            skip_group_check=True)

                # normalize + bias + output projection
                rb = ps.tile([32, 1024], FP32, tag="stg")
                for h in range(HPC):
                    hqs = slice(h * LQ + qc * 512, h * LQ + (qc + 1) * 512)
                    nc.vector.reciprocal(recip[32:33, hqs], av[h][32:33, :])
                    nc.tensor.matmul(rb[0:32, h * 512:(h + 1) * 512],
                                     ones[32:33, 0:32], recip[32:33, hqs],
                                     start=True, stop=True,
                                     tile_position=(32, 0))
                rbs = wkp.tile([32, 1024], FP32, tag="rbs")
                nc.vector.tensor_copy(rbs, rb)
                tmp = wkp.tile([32, 1024], FP32, tag="ctmp")
                for h in range(HPC):
                    hqs = slice(h * LQ + qc * 512, h * LQ + (qc + 1) * 512)
                    nc.vector.tensor_mul(tmp[:, h * 512:(h + 1) * 512],
                                         av[h][0:32, :],
                                         rbs[:, h * 512:(h + 1) * 512])
                    nc.vector.tensor_scalar_add(attnT[0:32, hqs],
                                                tmp[:, h * 512:(h + 1) * 512],
                                                bv_sb[:, h:h + 1])

                for ec in range(2):
                    po = ps.tile([128, 512], FP32, tag="stg")
                    for h in range(HPC):
                        hqs = slice(h * LQ + qc * 512, h * LQ + (qc + 1) * 512)
                        nc.tensor.matmul(
                            po, wo_r[:, h, ec * 128:(ec + 1) * 128],
                            attnT[0:32, hqs], start=(h == 0), stop=(h == 1))
                    nc.vector.tensor_copy(out_sb[ec][:, qs], po)

            for ec in range(2):
                nc.sync.dma_start(out=OUT[ec * 128:(ec + 1) * 128, :],
                                  in_=out_sb[ec])

    nc.compile()
    return nc


def _get_nc():
    if "nc" not in _CACHE:
        _CACHE["nc"] = _build()
    return _CACHE["nc"]


def kernel(bev_emb, queries, Wq, bq, Wk, bk, Wv, bv, Wo, bo):
    from concourse.bass_utils import run_bass_kernel_spmd

    bev_emb = np.asarray(bev_emb, dtype=np.float32)
    queries = np.asarray(queries, dtype=np.float32)
    Wq = np.asarray(Wq, dtype=np.float32)
    bq = np.asarray(bq, dtype=np.float32)
    Wk = np.asarray(Wk, dtype=np.float32)
    bk = np.asarray(bk, dtype=np.float32)
    Wv = np.asarray(Wv, dtype=np.float32)
    bv = np.asarray(bv, dtype=np.float32)
    Wo = np.asarray(Wo, dtype=np.float32)
    bo = np.asarray(bo, dtype=np.float32)

    xk_pad = np.zeros((B, LKP, E), dtype=np.float32)
    xk_pad[:, :LK, :] = bev_emb
    ident = np.eye(128, dtype=np.float32)

    in_maps = []
    for c in range(8):
        b, hp = c // 4, c % 4
        hs = slice(hp * DC, (hp + 1) * DC)
        in_maps.append({
            "xq": np.ascontiguousarray(queries[b]),
            "xk": np.ascontiguousarray(xk_pad[b]),
            "wq": np.ascontiguousarray(Wq[:, hs]),
            "wk": np.ascontiguousarray(Wk[:, hs]),
            "wv": np.ascontiguousarray(Wv[:, hs]),
            "wo": np.ascontiguousarray(Wo[hs, :]),
            "bq": np.ascontiguousarray(bq[hs]),
            "bk": np.ascontiguousarray(bk[hs]),
            "bv": np.ascontiguousarray(bv[hs]),
            "ident": ident,
        })

    nc = _get_nc()
    _CACHE["last_in_maps"] = in_maps
    res = run_bass_kernel_spmd(nc, in_maps, list(range(8)))
    _CACHE["last_result"] = res

    out = np.zeros((B, LQ, E), dtype=np.float32)
    for c in range(8):
        out[c // 4] += res.results[c]["out_t"].T
    out += bo
    return out


# revision 11
# speedup vs baseline: 1.0865x; 1.0783x over previous
"""Cross-attention kernel for Trainium2, SPMD over 8 NeuronCores.

Problem: B=2, LQ=1024, LK=10000, E=256, H=8 heads of D=32.
  q = queries @ Wq + bq ; k = bev @ Wk + bk ; v = bev @ Wv + bv
  out = softmax(q k^T) v  @ Wo + bo

Sharding: core c -> (batch b = c // 4, head-pair hp = c % 4).  Each core
computes attention for its 2 heads of its batch plus the partial output
projection through its 64 rows of Wo.  Host sums the 4 partials per batch
and adds bo (the gather/unshard step).  No collectives.

Numerics: big matmuls run as float32r (full-rate fp32, ~2^-11 rounding on
inputs, fp32 accumulate).  Softmax skips the max-subtraction (energies are
~N(0,32); exp stays finite in fp32) and the denominator is produced by an
extra all-ones column appended to v, so it falls out of the same PE matmul
that computes attn @ v.  bv is mathematically equivalent to a constant
added after normalization (softmax weights sum to 1), so v is projected
without bias and bv is added at the end.
"""
import sys

sys.path.insert(0, "/opt/trn_rl_repo")

import numpy as np

B, LQ, LK, E, H = 2, 1024, 10000, 256, 8
D = 32            # head dim
HPC = 2           # heads per core
DC = D * HPC      # 64 projected dims per core
LKP = 10240       # LK padded to a multiple of 128
NKT = LKP // 128  # 80 k-tiles
NCH = LKP // 512  # 20 dma/transpose chunks
GRP = 3           # (kt, head) units per exp instruction ([128, 1536] staging)

_CACHE = {}


def _build():
    import concourse.bacc as bacc
    import concourse.tile as tile
    from concourse import mybir

    FP32 = mybir.dt.float32
    FP32R = mybir.dt.float32r
    BF16 = mybir.dt.bfloat16
    AF = mybir.ActivationFunctionType

    nc = bacc.Bacc("TRN2", target_bir_lowering=False)

    XQ = nc.dram_tensor("xq", [LQ, E], FP32, kind="ExternalInput")
    XK = nc.dram_tensor("xk", [LKP, E], FP32, kind="ExternalInput")
    WQ = nc.dram_tensor("wq", [E, DC], FP32, kind="ExternalInput")
    WK = nc.dram_tensor("wk", [E, DC], FP32, kind="ExternalInput")
    WV = nc.dram_tensor("wv", [E, DC], FP32, kind="ExternalInput")
    WO = nc.dram_tensor("wo", [DC, E], FP32, kind="ExternalInput")
    BQ = nc.dram_tensor("bq", [DC], FP32, kind="ExternalInput")
    BK = nc.dram_tensor("bk", [DC], FP32, kind="ExternalInput")
    BV = nc.dram_tensor("bv", [DC], FP32, kind="ExternalInput")
    IDT = nc.dram_tensor("ident", [128, 128], FP32, kind="ExternalInput")
    # partial output, transposed: rows = embed dim, cols = query position
    OUT = nc.dram_tensor("out_t", [E, LQ], FP32, kind="ExternalOutput")

    with tile.TileContext(nc) as tc:
        with (
            tc.tile_pool(name="singles", bufs=1) as sg,
            tc.tile_pool(name="aio", bufs=3) as aio,
            tc.tile_pool(name="wk", bufs=3) as wkp,
            tc.tile_pool(name="ps", bufs=2, space="PSUM") as ps,
            tc.tile_pool(name="av", bufs=2, space="PSUM") as avp,
        ):
            # ---- constants ----
            ident = sg.tile([128, 128], FP32, tag="ident")
            nc.sync.dma_start(out=ident, in_=IDT[:, :])

            ones = sg.tile([128, 160], FP32, tag="ones")
            nc.vector.memset(ones, 1.0)

            def load_round(dram_ap, shape, tag):
                f = aio.tile(shape, FP32, tag="wstage", name="wstage")
                nc.sync.dma_start(out=f, in_=dram_ap)
                r = sg.tile(shape, FP32R, tag=tag, name=tag)
                nc.vector.tensor_copy(r, f)
                return r

            wq_r = load_round(WQ[:, :].rearrange("(c p) m -> p c m", p=128),
                              [128, 2, DC], "wq")
            wk_r = load_round(WK[:, :].rearrange("(c p) m -> p c m", p=128),
                              [128, 2, DC], "wk")
            wv_r = load_round(WV[:, :].rearrange("(c p) m -> p c m", p=128),
                              [128, 2, DC], "wv")

            # Wo rows: head h's 32 rows at partitions 0-31, column block h
            wo_f = aio.tile([32, 2, E], FP32, tag="wstage2")
            nc.sync.dma_start(
                out=wo_f, in_=WO[:, :].rearrange("(a p) m -> p a m", p=32))
            wo_r = sg.tile([32, 2, E], FP32R, tag="wo")
            nc.vector.tensor_copy(wo_r, wo_f)

            bq_sb = sg.tile([64, 1], FP32, tag="bq")
            nc.sync.dma_start(out=bq_sb, in_=BQ[:].rearrange("(p o) -> p o", o=1))
            bk_sb = sg.tile([64, 1], FP32, tag="bk")
            nc.sync.dma_start(out=bk_sb, in_=BK[:].rearrange("(p o) -> p o", o=1))
            # bv: column h holds head h's 32 bias values on partitions 0-31
            bv_sb = sg.tile([32, 2], FP32, tag="bv")
            nc.sync.dma_start(out=bv_sb,
                              in_=BV[:].rearrange("(a p) -> p a", p=32))

            # ---- stage A-q: transpose queries, project q^T ----
            xqT = [sg.tile([128, LQ], FP32R, tag=f"xqT{e}", name=f"xqT{e}") for e in range(2)]
            for half in range(2):          # groups of 4 q-tiles
                pts = [ps.tile([128, 512], FP32, tag="stg", name=f"ptsq{half}{i}") for i in range(2)]
                for j in range(4):
                    qt = half * 4 + j
                    xq_t = aio.tile([128, E], FP32, tag="xq")
                    nc.sync.dma_start(out=xq_t, in_=XQ[qt * 128:(qt + 1) * 128, :])
                    for e in range(2):
                        nc.tensor.transpose(
                            pts[e][:, j * 128:(j + 1) * 128],
                            xq_t[:, e * 128:(e + 1) * 128], ident)
                for e in range(2):
                    nc.vector.tensor_copy(
                        xqT[e][:, half * 512:(half + 1) * 512], pts[e])

            # qT rows 0-63 = heads {h0, h1}; rows 64-127 = a copy, so the
            # energy matmuls can run 3-at-a-time in distinct PE row groups.
            qT = sg.tile([128, LQ], FP32R, tag="qT")
            for qc in range(2):
                qp = ps.tile([64, 512], FP32, tag="stg")
                for e in range(2):
                    nc.tensor.matmul(qp, wq_r[:, e, :],
                                     xqT[e][:, qc * 512:(qc + 1) * 512],
                                     start=(e == 0), stop=(e == 1))
                nc.vector.tensor_scalar_add(
                    qT[0:64, qc * 512:(qc + 1) * 512], qp, bq_sb[:, 0:1])
            nc.sync.dma_start(out=qT[64:128, :], in_=qT[0:64, :])

            # ---- stage A-k: transpose bev chunks, project k^T and v ----
            # kT rows 0-63 = heads {h0, h1}; rows 64-127 = copy (for 3x row
            # packing of the energy matmuls).
            kT = sg.tile([128, LKP], FP32R, tag="kT")
            v_aug = sg.tile([128, NKT * 66], BF16, tag="vaug")
            # ones columns of v_aug (the softmax-denominator trick)
            nc.vector.tensor_copy(
                v_aug[:, :].rearrange("p (k o) -> p k o", o=33)[:, :, 32:33],
                ones[:, :].rearrange("p (k o) -> p k o", o=1))

            # attnT: head h's normalized attn output (32 dims) on partitions
            # 0-31, columns h*LQ + q.  recip row 32 holds 1/denominator.
            attnT = sg.tile([32, 2 * LQ], FP32R, tag="attnT")
            recip = sg.tile([33, 2 * LQ], FP32, tag="recip")
            out_sb = [sg.tile([128, LQ], FP32, tag=f"out{e}", name=f"out{e}") for e in range(2)]

            av = {}
            n_grp = [0, 0]

            def emit_group(grp, qc):
                # 3 energy matmuls in distinct PE row groups (concurrent),
                # one big exp, then the attn@v_aug accumulations.
                qs = slice(qc * 512, (qc + 1) * 512)
                stg = ps.tile([128, 512 * len(grp)], FP32, tag="stg",
                              name=f"stg{qc}_{n_grp[qc]}")
                for i, (kt, h) in enumerate(grp):
                    row = 32 * h if i < 2 else 64 + 32 * h
                    nc.tensor.matmul(
                        stg[:, i * 512:(i + 1) * 512],
                        kT[row:row + 32, kt * 128:(kt + 1) * 128],
                        qT[row:row + 32, qs],
                        start=True, stop=True, tile_position=(row, 0))
                sT = wkp.tile([128, 512 * len(grp)], BF16, tag="sT",
                              name=f"sT{qc}_{n_grp[qc]}")
                nc.scalar.activation(sT, stg, AF.Exp)
                for i, (kt, h) in enumerate(grp):
                    nc.tensor.matmul(
                        av[qc][h][0:33, :],
                        v_aug[:, kt * 66 + 33 * h:kt * 66 + 33 * h + 33],
                        sT[:, i * 512:(i + 1) * 512],
                        start=(kt == 0), stop=(kt == NKT - 1),
                        skip_group_check=True)
                n_grp[qc] += 1

            av[0] = [avp.tile([33, 512], FP32, tag="av", name=f"av0{h}")
                     for h in range(HPC)]
            pending = []
            for c in range(NCH):
                xk_t = aio.tile([128, 4, E], FP32, tag="xk")
                nc.sync.dma_start(
                    out=xk_t,
                    in_=XK[c * 512:(c + 1) * 512, :].rearrange(
                        "(t p) e -> p t e", p=128))
                xkT = []
                for e in range(2):
                    pt = ps.tile([128, 512], FP32, tag="stg")
                    for t in range(4):
                        nc.tensor.transpose(
                            pt[:, t * 128:(t + 1) * 128],
                            xk_t[:, t, e * 128:(e + 1) * 128], ident)
                    xe = wkp.tile([128, 512], FP32R, tag=f"xkT{e}")
                    nc.vector.tensor_copy(xe, pt)
                    xkT.append(xe)

                kp = ps.tile([64, 512], FP32, tag="stg")
                for e in range(2):
                    nc.tensor.matmul(kp, wk_r[:, e, :], xkT[e],
                                     start=(e == 0), stop=(e == 1))
                nc.vector.tensor_scalar_add(
                    kT[0:64, c * 512:(c + 1) * 512], kp, bk_sb[:, 0:1])
                nc.sync.dma_start(out=kT[64:128, c * 512:(c + 1) * 512],
                                  in_=kT[0:64, c * 512:(c + 1) * 512])

                vp = ps.tile([64, 512], FP32, tag="stg")
                for e in range(2):
                    nc.tensor.matmul(vp, wv_r[:, e, :], xkT[e],
                                     start=(e == 0), stop=(e == 1))
                vt = wkp.tile([64, 512], FP32, tag="vt")
                nc.vector.tensor_copy(vt, vp)

                vps = ps.tile([128, 256], FP32, tag="stg")
                for t in range(4):
                    nc.tensor.transpose(vps[:, t * 64:(t + 1) * 64],
                                        vt[:, t * 128:(t + 1) * 128],
                                        ident[0:64, 0:64])
                nc.vector.tensor_copy(
                    v_aug[:, c * 264:(c + 1) * 264].rearrange(
                        "p (t a b) -> p t a b", a=2, b=33)[:, :, :, 0:32],
                    vps[:, :].rearrange("p (t a b) -> p t a b", a=2, b=32))

                # interleave: emit ready qc=0 attention groups for this chunk
                pending += [(kt, h) for kt in range(4 * c, 4 * c + 4)
                            for h in range(HPC)]
                while len(pending) >= GRP:
                    emit_group(pending[:GRP], 0)
                    pending = pending[GRP:]

            if pending:
                emit_group(pending, 0)

            def stage_c(qc):
                qs = slice(qc * 512, (qc + 1) * 512)
                # normalize + bias + output projection
                rb = ps.tile([32, 1024], FP32, tag="stg", name=f"rb{qc}")
                for h in range(HPC):
                    hqs = slice(h * LQ + qc * 512, h * LQ + (qc + 1) * 512)
                    nc.vector.reciprocal(recip[32:33, hqs], av[qc][h][32:33, :])
                    nc.tensor.matmul(rb[0:32, h * 512:(h + 1) * 512],
                                     ones[32:33, 0:32], recip[32:33, hqs],
                                     start=True, stop=True,
                                     tile_position=(32, 0))
                rbs = wkp.tile([32, 1024], FP32, tag="rbs", name=f"rbs{qc}")
                nc.vector.tensor_copy(rbs, rb)
                tmp = wkp.tile([32, 1024], FP32, tag="ctmp", name=f"ctmp{qc}")
                for h in range(HPC):
                    hqs = slice(h * LQ + qc * 512, h * LQ + (qc + 1) * 512)
                    nc.vector.tensor_mul(tmp[:, h * 512:(h + 1) * 512],
                                         av[qc][h][0:32, :],
                                         rbs[:, h * 512:(h + 1) * 512])
                    nc.vector.tensor_scalar_add(attnT[0:32, hqs],
                                                tmp[:, h * 512:(h + 1) * 512],
                                                bv_sb[:, h:h + 1])

                for ec in range(2):
                    po = ps.tile([128, 512], FP32, tag="stg", name=f"po{qc}{ec}")
                    for h in range(HPC):
                        hqs = slice(h * LQ + qc * 512, h * LQ + (qc + 1) * 512)
                        nc.tensor.matmul(
                            po, wo_r[:, h, ec * 128:(ec + 1) * 128],
                            attnT[0:32, hqs], start=(h == 0), stop=(h == 1))
                    nc.vector.tensor_copy(out_sb[ec][:, qs], po)

            stage_c(0)
            av[1] = [avp.tile([33, 512], FP32, tag="av", name=f"av1{h}")
                     for h in range(HPC)]
            units = [(kt, h) for kt in range(NKT) for h in range(HPC)]
            for g0 in range(0, len(units), GRP):
                emit_group(units[g0:g0 + GRP], 1)
            stage_c(1)

            for ec in range(2):
                nc.sync.dma_start(out=OUT[ec * 128:(ec + 1) * 128, :],
                                  in_=out_sb[ec])

    nc.compile()
    return nc


def _get_nc():
    if "nc" not in _CACHE:
        _CACHE["nc"] = _build()
    return _CACHE["nc"]


def kernel(bev_emb, queries, Wq, bq, Wk, bk, Wv, bv, Wo, bo):
    from concourse.bass_utils import run_bass_kernel_spmd

    bev_emb = np.asarray(bev_emb, dtype=np.float32)
    queries = np.asarray(queries, dtype=np.float32)
    Wq = np.asarray(Wq, dtype=np.float32)
    bq = np.asarray(bq, dtype=np.float32)
    Wk = np.asarray(Wk, dtype=np.float32)
    bk = np.asarray(bk, dtype=np.float32)
    Wv = np.asarray(Wv, dtype=np.float32)
    bv = np.asarray(bv, dtype=np.float32)
    Wo = np.asarray(Wo, dtype=np.float32)
    bo = np.asarray(bo, dtype=np.float32)

    xk_pad = np.zeros((B, LKP, E), dtype=np.float32)
    xk_pad[:, :LK, :] = bev_emb
    ident = np.eye(128, dtype=np.float32)

    in_maps = []
    for c in range(8):
        b, hp = c // 4, c % 4
        hs = slice(hp * DC, (hp + 1) * DC)
        in_maps.append({
            "xq": np.ascontiguousarray(queries[b]),
            "xk": np.ascontiguousarray(xk_pad[b]),
            "wq": np.ascontiguousarray(Wq[:, hs]),
            "wk": np.ascontiguousarray(Wk[:, hs]),
            "wv": np.ascontiguousarray(Wv[:, hs]),
            "wo": np.ascontiguousarray(Wo[hs, :]),
            "bq": np.ascontiguousarray(bq[hs]),
            "bk": np.ascontiguousarray(bk[hs]),
            "bv": np.ascontiguousarray(bv[hs]),
            "ident": ident,
        })

    nc = _get_nc()
    _CACHE["last_in_maps"] = in_maps
    res = run_bass_kernel_spmd(nc, in_maps, list(range(8)))
    _CACHE["last_result"] = res

    out = np.zeros((B, LQ, E), dtype=np.float32)
    for c in range(8):
        out[c // 4] += res.results[c]["out_t"].T
    out += bo
    return out
